# revision 1
# baseline (speedup 1.0000x reference)
"""Baichuan transformer layer on 8 Trainium2 NeuronCores (Megatron TP-8).

Dataflow (per core, SPMD):
  - activations live transposed ([feature, token]) so every matmul takes
    natural-layout weights as the stationary operand;
  - column-shard w_pack/gate/up, row-shard o_proj/down, 5 heads per core;
  - bf16 matmuls, fp32 softmax/norm/residual chains;
  - AllReduce after o_proj, ReduceScatter after down_proj, both bf16 and
    sequence-chunked so comm overlaps compute.

Host side: folds RMSNorm weights into w_pack/gate/up rows, folds the
1/sqrt(HD) attention scale into the q columns, pre-transposes
hidden_states and attention_mask, shards, runs the NEFF, reassembles.

The device returns (a) the full post-attention residual h2 = hidden +
attn_out (identical on every core) and (b) each core's ReduceScatter
shard of the MLP delta; the host does out = h2_shard + delta_shard and
un-transposes.  (The residual add can't be done on device: the shard's
row offset is rank-dependent and the SPMD graph is identical per core.)
"""

import math

import numpy as np
import ml_dtypes

import concourse.bass as bass
import concourse.mybir as mybir
import concourse.tile as tile
from concourse import bacc
from concourse.bass_utils import run_bass_kernel_spmd
from concourse.masks import make_identity
from concourse.alu_op_type import AluOpType
import concourse.bass_isa as bass_isa

F32 = mybir.dt.float32
F32R = mybir.dt.float32r
BF16 = mybir.dt.bfloat16
NPBF16 = ml_dtypes.bfloat16

N_CORES = 8
S = 1024          # tokens
H = 5120          # hidden
HK = H // 128     # 40 hidden k-tiles
NH = 40           # heads total
NH_SH = NH // N_CORES   # 5 heads per core
HD = 128          # head dim
F = NH_SH * HD    # 640 attn features per core
INTER = 13696
ISH = INTER // N_CORES  # 1712 inter features per core
IK = (ISH + 127) // 128  # 14 inter k-tiles (last = 48 rows)
EPS = 1e-6

CH = 2                 # comm (AllReduce) chunks
W = S // CH            # tokens per comm chunk (512)
MH = 2                 # MLP halves
WM = S // MH           # tokens per MLP half (512)
ST = S // 128          # 8 token 128-tiles

QKV_GRP = 3            # m-chunks per psum group (x2 s-halves = 6 banks)
OP_GRP = 3             # o_proj c-chunks per group
GU_GRP = 3             # gate/up m-chunks per group
DN_GRP = 6             # down c-chunks per group



def build_nc():
    nc = bacc.Bacc("TRN2", target_bir_lowering=False, debug=False,
                   num_devices=N_CORES)

    # ---- I/O ----
    hT = nc.dram_tensor("hT", [H, S], F32, kind="ExternalInput")
    maskT = nc.dram_tensor("maskT", [NH_SH, S, S], F32, kind="ExternalInput")
    wp = nc.dram_tensor("wp", [H, 3 * F], BF16, kind="ExternalInput")
    wo = nc.dram_tensor("wo", [F, H], BF16, kind="ExternalInput")
    wg = nc.dram_tensor("wg", [H, ISH], BF16, kind="ExternalInput")
    wu = nc.dram_tensor("wu", [H, ISH], BF16, kind="ExternalInput")
    wd = nc.dram_tensor("wd", [ISH, H], BF16, kind="ExternalInput")
    out = nc.dram_tensor("out", [F, S], F32, kind="ExternalOutput")
    h2o = nc.dram_tensor("h2o", [H, S], F32, kind="ExternalOutput")

    # ---- internal DRAM (collective bounce buffers) ----
    ar_in = [nc.dram_tensor(f"ar_in{c}", [H, W], BF16) for c in range(CH)]
    ar_out = [nc.dram_tensor(f"ar_out{c}", [H, W], BF16, addr_space="Shared")
              for c in range(CH)]
    rs_in = [nc.dram_tensor(f"rs_in{c}", [H, WM], BF16) for c in range(MH)]
    rs_out = [nc.dram_tensor(f"rs_out{c}", [F, WM], BF16) for c in range(MH)]

    with tile.TileContext(nc) as tc:
        with (
            tc.tile_pool(name="const", bufs=1) as constp,
            tc.tile_pool(name="ps", bufs=6, space="PSUM") as psp,
            tc.tile_pool(name="tp_ps", bufs=2, space="PSUM") as tpps,
        ):
            ones_f32 = constp.tile([128, 1], F32, tag="ones_f32")
            nc.any.memset(ones_f32[:], 1.0)
            ones_f = constp.tile([128, 1], F32R, tag="ones_f")
            nc.vector.tensor_copy(ones_f[:], ones_f32[:])
            ones_b = constp.tile([128, 1], BF16, tag="ones_b")
            nc.any.memset(ones_b[:], 1.0)
            onesr_f32 = constp.tile([1, 128], F32, tag="onesr_f32")
            nc.any.memset(onesr_f32[:], 1.0)
            onesr_f = constp.tile([1, 128], F32R, tag="onesr_f")
            nc.vector.tensor_copy(onesr_f[:], onesr_f32[:])
            ident_b = constp.tile([128, 128], BF16, tag="ident_b")
            make_identity(nc, ident_b)
            
            aop = tc.alloc_tile_pool(name="ao_pool", bufs=1)
            aoT = [aop.tile([128, S], BF16, tag=f"aoT{h}", name=f"aoT{h}")
                   for h in range(NH_SH)]
            qkp = tc.alloc_tile_pool(name="qk_pool", bufs=1)
            qT = [qkp.tile([128, S], BF16, tag=f"qT{h}", name=f"qT{h}")
                  for h in range(NH_SH)]
            kT = [qkp.tile([128, S], BF16, tag=f"kT{h}", name=f"kT{h}")
                  for h in range(NH_SH)]
            vn = [qkp.tile([128, F], BF16, tag=f"vn{s}", name=f"vn{s}")
                  for s in range(ST)]

            # X = rms_norm(h)^T in bf16, resident in SBUF through QKV
            xpool = tc.alloc_tile_pool(name="xpool", bufs=1)
            X = [xpool.tile([128, S], BF16, tag=f"x{k}", name=f"x{k}")
                 for k in range(HK)]

            # ================= phase 0: rms scale1 + X =================
            p0pool = tc.alloc_tile_pool(name="p0", bufs=4)
            sc1b = p0pool.tile([128, S], F32, tag="sc1b", bufs=1)
            ss_ps = [psp.tile([1, 512], F32, tag="ps", name=f"ss_ps{i}")
                     for i in range(2)]
            for k in range(HK):
                t = p0pool.tile([128, S], F32, tag="ht_in")
                nc.sync.dma_start(out=t[:], in_=hT[k * 128:(k + 1) * 128, :])
                sq = p0pool.tile([128, S], F32R, tag="sq", bufs=3)
                nc.vector.tensor_mul(sq[:], t[:], t[:])
                for half in range(2):
                    nc.tensor.matmul(
                        ss_ps[half][:], ones_f[:],
                        sq[:, half * 512:(half + 1) * 512],
                        start=(k == 0), stop=(k == HK - 1))
            s1row = constp.tile([1, S], F32, tag="s1row")
            for half in range(2):
                hs = slice(half * 512, (half + 1) * 512)
                nc.vector.tensor_scalar(
                    s1row[:, hs], ss_ps[half][:], 1.0 / H, EPS,
                    AluOpType.mult, AluOpType.add)
            s1r2 = constp.tile([1, S], F32, tag="s1r2")
            nc.vector.reciprocal(s1r2[:], s1row[:])
            s1r3 = constp.tile([1, S], F32R, tag="s1r3")
            with nc.allow_low_precision(reason="fp32r scale row"):
                nc.scalar.sqrt(s1r3[:], s1r2[:])      # rsqrt(mean+eps)
            for half in range(2):
                hs = slice(half * 512, (half + 1) * 512)
                bps = psp.tile([128, 512], F32, tag="ps")
                nc.tensor.matmul(bps[:], onesr_f[:], s1r3[:, hs],
                                 start=True, stop=True)
                nc.scalar.copy(sc1b[:, hs], bps[:])
            for k in range(HK):
                t = p0pool.tile([128, S], F32, tag="ht_in")
                nc.sync.dma_start(out=t[:], in_=hT[k * 128:(k + 1) * 128, :])
                nc.vector.tensor_mul(X[k][:], t[:], sc1b[:])
            p0pool.release()

            # ================= phase 1: QKV (q^T, k^T, v nat) ===========
            qkvstr = tc.alloc_tile_pool(name="qkvstr", bufs=3)
            n_mch = 3 * NH_SH  # 15 col chunks of the w_pack shard
            for g0 in range(0, n_mch, QKV_GRP):
                gsz = min(QKV_GRP, n_mch - g0)
                pst = [[psp.tile([128, 512], F32, tag="ps",
                                 name=f"qkvps{mi}_{half}")
                        for half in range(2)] for mi in range(gsz)]
                for k in range(HK):
                    wsl = qkvstr.tile([128, QKV_GRP * 128], BF16, tag="wp_sl")
                    nc.sync.dma_start(
                        out=wsl[:, :gsz * 128],
                        in_=wp[k * 128:(k + 1) * 128,
                               g0 * 128:(g0 + gsz) * 128])
                    for mi in range(gsz):
                        for half in range(2):
                            nc.tensor.matmul(
                                pst[mi][half][:],
                                wsl[:, mi * 128:(mi + 1) * 128],
                                X[k][:, half * 512:(half + 1) * 512],
                                start=(k == 0), stop=(k == HK - 1))
                for mi in range(gsz):
                    m = g0 + mi
                    for half in range(2):
                        hs = slice(half * 512, (half + 1) * 512)
                        if m < NH_SH:
                            nc.scalar.copy(qT[m][:, hs], pst[mi][half][:])
                        elif m < 2 * NH_SH:
                            nc.scalar.copy(kT[m - NH_SH][:, hs],
                                           pst[mi][half][:])
                        else:
                            h = m - 2 * NH_SH
                            vt = qkvstr.tile([128, 512], BF16, tag="vT_ev")
                            nc.scalar.copy(vt[:], pst[mi][half][:])
                            for sb in range(4):
                                s_tile = half * 4 + sb
                                tps = tpps.tile([128, 128], BF16, tag="tp")
                                nc.tensor.transpose(
                                    tps[:], vt[:, sb * 128:(sb + 1) * 128],
                                    ident_b[:])
                                nc.scalar.copy(
                                    vn[s_tile][:, h * 128:(h + 1) * 128],
                                    tps[:])
            qkvstr.release()
            xpool.release()

            # == phase 2+3: attention / o_proj / AR / h2 / Y, i-chunked ==
            chstr = tc.alloc_tile_pool(name="chstr", bufs=3)
            mlpp = tc.alloc_tile_pool(name="mlp", bufs=1)
            yts = [None] * MH
            expp = tc.alloc_tile_pool(name="exp_pool", bufs=18)
            attnstr = tc.alloc_tile_pool(name="attnstr", bufs=3)
            for c in range(CH):
                ci = slice(c * W, (c + 1) * W)

                def emit_scores(h, ci=ci):
                    expT = []
                    for j in range(ST):
                        mk = attnstr.tile([128, W], F32, tag="mask_in",
                                          bufs=3, name="mk")
                        nc.sync.dma_start(
                            out=mk[:], in_=maskT[h, j * 128:(j + 1) * 128, ci])
                        scf = attnstr.tile([128, W], F32, tag="sc_f",
                                           bufs=3, name="scf")
                        sps = psp.tile([128, W], F32, tag="ps", name="sps")
                        nc.tensor.matmul(
                            sps[:], kT[h][:, j * 128:(j + 1) * 128],
                            qT[h][:, ci], start=True, stop=True)
                        nc.vector.tensor_add(scf[:], sps[:], mk[:])
                        et = expp.tile([128, W], BF16, tag="expT", name="et")
                        nc.scalar.activation(
                            et[:], scf[:], mybir.ActivationFunctionType.Exp)
                        expT.append(et)
                    return expT

                def emit_post(h, expT, ci=ci):
                    lt = [attnstr.tile([128, W], F32, tag=f"ltree{i}",
                                       name=f"ltree{i}", bufs=1)
                          for i in range(3)]
                    nc.vector.tensor_add(lt[0][:], expT[0][:], expT[1][:])
                    nc.vector.tensor_add(lt[1][:], expT[2][:], expT[3][:])
                    nc.vector.tensor_add(lt[2][:], expT[4][:], expT[5][:])
                    nc.vector.tensor_add(lt[0][:], lt[0][:], lt[1][:])
                    lt1b = attnstr.tile([128, W], F32, tag="ltree1",
                                        name="lt1b", bufs=1)
                    nc.vector.tensor_add(lt1b[:], expT[6][:], expT[7][:])
                    nc.vector.tensor_add(lt[2][:], lt[2][:], lt1b[:])
                    l7 = attnstr.tile([128, W], F32R, tag="l7", bufs=1)
                    nc.vector.tensor_add(l7[:], lt[0][:], lt[2][:])
                    l_ps = psp.tile([1, W], F32, tag="ps", name="l_ps")
                    nc.tensor.matmul(l_ps[:], ones_f[:], l7[:],
                                     start=True, stop=True)
                    inv = attnstr.tile([1, W], F32R, tag="inv_l", bufs=1)
                    with nc.allow_low_precision(reason="f32r inv"):
                        nc.vector.reciprocal(inv[:], l_ps[:])
                    ibp = psp.tile([128, W], F32, tag="ps", name="ibp")
                    nc.tensor.matmul(ibp[:], onesr_f[:], inv[:],
                                     start=True, stop=True)
                    ibs = attnstr.tile([128, W], F32, tag="ib_s", bufs=1)
                    nc.scalar.copy(ibs[:], ibp[:])
                    avp = psp.tile([128, W], F32, tag="ps", name="avp")
                    for j in range(ST):
                        nc.tensor.matmul(
                            avp[:], vn[j][:, h * 128:(h + 1) * 128],
                            expT[j][:], start=(j == 0), stop=(j == ST - 1))
                    nc.vector.tensor_mul(aoT[h][:, ci], avp[:], ibs[:])

                prev = emit_scores(0)
                for h in range(1, NH_SH):
                    cur = emit_scores(h)
                    emit_post(h - 1, prev)
                    prev = cur
                emit_post(NH_SH - 1, prev)

                # ---- o_proj partials for chunk c -> AllReduce c ----
                for g0 in range(0, HK, OP_GRP):
                    gsz = min(OP_GRP, HK - g0)
                    pst = [psp.tile([128, W], F32, tag="ps", name=f"ops{mi}")
                           for mi in range(gsz)]
                    for f in range(NH_SH):
                        wsl = attnstr.tile([128, OP_GRP * 128], BF16,
                                           tag="wo_sl")
                        nc.sync.dma_start(
                            out=wsl[:, :gsz * 128],
                            in_=wo[f * 128:(f + 1) * 128,
                                   g0 * 128:(g0 + gsz) * 128])
                        for mi in range(gsz):
                            nc.tensor.matmul(
                                pst[mi][:], wsl[:, mi * 128:(mi + 1) * 128],
                                aoT[f][:, ci],
                                start=(f == 0), stop=(f == NH_SH - 1))
                    for mi in range(gsz):
                        m = g0 + mi
                        ob = attnstr.tile([128, W], BF16, tag="o_ev", bufs=2)
                        nc.scalar.copy(ob[:], pst[mi][:])
                        nc.sync.dma_start(
                            out=ar_in[c][m * 128:(m + 1) * 128, :], in_=ob[:])
                nc.gpsimd.collective_compute(
                    "AllReduce", mybir.AluOpType.add,
                    ins=[ar_in[c][:, :].opt()], outs=[ar_out[c][:, :].opt()],
                    replica_groups=[list(range(N_CORES))])

            attnstr.release()
            expp.release()

            # ===== phase 5: per chunk: h2/ln2/Y then MLP + ReduceScatter ==
            for mh in range(MH):
                ms = slice(mh * WM, (mh + 1) * WM)
                c = mh
                ci = ms
                # ---- h2 = hT + ar (stream to h2o), ln2 stats, Y chunk ----
                ss2 = [psp.tile([1, W], F32, tag="ps", name=f"ss2_{i}")
                       for i in range(2)]
                for k in range(HK):
                    ht = chstr.tile([128, W], F32, tag="ht2_in", bufs=2)
                    nc.gpsimd.dma_start(
                        out=ht[:], in_=hT[k * 128:(k + 1) * 128, ci])
                    arb = chstr.tile([128, W], BF16, tag="ar_b", bufs=2)
                    nc.gpsimd.dma_start(
                        out=arb[:], in_=ar_out[c][k * 128:(k + 1) * 128, :])
                    arf = chstr.tile([128, W], F32, tag="ar_f", bufs=2)
                    nc.scalar.copy(arf[:], arb[:])
                    h2t = chstr.tile([128, W], F32, tag="h2t", bufs=2)
                    nc.vector.tensor_add(h2t[:], ht[:], arf[:])
                    nc.gpsimd.dma_start(
                        out=h2o[k * 128:(k + 1) * 128, ci], in_=h2t[:])
                    sq = chstr.tile([128, W], F32R, tag="sq2", bufs=2)
                    nc.vector.tensor_mul(sq[:], h2t[:], h2t[:])
                    nc.tensor.matmul(ss2[k % 2][:], ones_f[:], sq[:],
                                     start=(k < 2), stop=(k >= HK - 2))
                ss2c = chstr.tile([1, W], F32, tag="ss2c", bufs=1)
                nc.scalar.copy(ss2c[:], ss2[1][:])
                s2a = chstr.tile([1, W], F32, tag="s2a", bufs=1)
                nc.vector.tensor_add(s2a[:], ss2[0][:], ss2c[:])
                nc.vector.tensor_scalar(s2a[:], s2a[:], 1.0 / H, EPS,
                                        AluOpType.mult, AluOpType.add)
                s2b = chstr.tile([1, W], F32, tag="s2b", bufs=1)
                nc.vector.reciprocal(s2b[:], s2a[:])
                s2c = chstr.tile([1, W], F32R, tag="s2c", bufs=1)
                with nc.allow_low_precision(reason="fp32r scale row"):
                    nc.scalar.sqrt(s2c[:], s2b[:])
                bps = psp.tile([128, W], F32, tag="ps", name="bps2")
                nc.tensor.matmul(bps[:], onesr_f[:], s2c[:],
                                 start=True, stop=True)
                sc2b = chstr.tile([128, W], F32, tag="sc2b", bufs=2)
                nc.scalar.copy(sc2b[:], bps[:])
                yts[mh] = [mlpp.tile([128, WM], BF16, tag=f"y_{k}",
                                     name=f"y_{k}") for k in range(HK)]
                for k in range(HK):
                    h2r = chstr.tile([128, W], F32, tag="ht2_in", bufs=2)
                    nc.gpsimd.dma_start(
                        out=h2r[:], in_=h2o[k * 128:(k + 1) * 128, ci])
                    nc.vector.tensor_mul(yts[mh][k][:], h2r[:], sc2b[:])
                # ---- gate/up (interleaved per group) ----
                gu = [mlpp.tile([128, WM], BF16, tag=f"gu_{m}",
                                name=f"gu_{m}") for m in range(IK)]
                for g0 in range(0, IK, GU_GRP):
                    gsz = min(GU_GRP, IK - g0)
                    gs = [mlpp.tile([128, WM], F32, tag=f"gs_{mi}",
                                    name=f"gs_{mi}")
                          for mi in range(gsz)]
                    for wgt_i, wgt in enumerate((wg, wu)):
                        pst = [psp.tile([128, WM], F32, tag="ps",
                                        name=f"gups{mi}") for mi in range(gsz)]
                        for k in range(HK):
                            wsl = chstr.tile([128, GU_GRP * 128], BF16,
                                             tag="gu_sl")
                            c0 = g0 * 128
                            c1 = min((g0 + gsz) * 128, ISH)
                            nc.sync.dma_start(
                                out=wsl[:, :c1 - c0],
                                in_=wgt[k * 128:(k + 1) * 128, c0:c1])
                            for mi in range(gsz):
                                mw = min(128, ISH - (g0 + mi) * 128)
                                nc.tensor.matmul(
                                    pst[mi][:mw, :],
                                    wsl[:, mi * 128:mi * 128 + mw],
                                    yts[mh][k][:],
                                    start=(k == 0), stop=(k == HK - 1))
                        for mi in range(gsz):
                            m = g0 + mi
                            mw = min(128, ISH - m * 128)
                            if wgt_i == 0:
                                nc.scalar.activation(
                                    gs[mi][:mw, :], pst[mi][:mw, :],
                                    mybir.ActivationFunctionType.Silu)
                            else:
                                nc.vector.tensor_mul(
                                    gu[m][:mw, :], pst[mi][:mw, :],
                                    gs[mi][:mw, :])

                # ---- down partial -> rs_in ----
                for g0 in range(0, HK, DN_GRP):
                    gsz = min(DN_GRP, HK - g0)
                    pst = [psp.tile([128, WM], F32, tag="ps",
                                    name=f"dps{mi}") for mi in range(gsz)]
                    for k in range(IK):
                        kw = min(128, ISH - k * 128)
                        wsl = chstr.tile([128, DN_GRP * 128], BF16,
                                         tag="dn_sl")
                        nc.sync.dma_start(
                            out=wsl[:kw, :gsz * 128],
                            in_=wd[k * 128:k * 128 + kw,
                                   g0 * 128:(g0 + gsz) * 128])
                        for mi in range(gsz):
                            nc.tensor.matmul(
                                pst[mi][:],
                                wsl[:kw, mi * 128:(mi + 1) * 128],
                                gu[k][:kw, :],
                                start=(k == 0), stop=(k == IK - 1))
                    for mi in range(gsz):
                        m = g0 + mi
                        db = chstr.tile([128, WM], BF16, tag="d_ev", bufs=2)
                        nc.scalar.copy(db[:], pst[mi][:])
                        nc.sync.dma_start(
                            out=rs_in[mh][m * 128:(m + 1) * 128, :],
                            in_=db[:])
                nc.gpsimd.collective_compute(
                    "ReduceScatter", mybir.AluOpType.add,
                    ins=[rs_in[mh][:, :].opt()],
                    outs=[rs_out[mh][:, :].opt()],
                    replica_groups=[list(range(N_CORES))])

                # ---- out = rs_out (delta shard) as f32 ----
                for k5 in range(F // 128):
                    rsb = chstr.tile([128, WM], BF16, tag="rs_b", bufs=2)
                    nc.gpsimd.dma_start(
                        out=rsb[:], in_=rs_out[mh][k5 * 128:(k5 + 1) * 128, :])
                    rsf = chstr.tile([128, WM], F32, tag="rs_f", bufs=2)
                    nc.scalar.copy(rsf[:], rsb[:])
                    nc.gpsimd.dma_start(
                        out=out[k5 * 128:(k5 + 1) * 128, ms], in_=rsf[:])
            mlpp.release()
            chstr.release()
            qkp.release()
            aop.release()

    nc.compile()
    return nc




_NC_CACHE = None


def _get_nc():
    global _NC_CACHE
    if _NC_CACHE is None:
        _NC_CACHE = build_nc()
    return _NC_CACHE


def prepare_in_maps(hidden_states, attention_mask, w_pack, o_proj, gate_proj,
                    up_proj, down_proj, ln1_w, ln2_w):
    hidden_states = np.asarray(hidden_states, dtype=np.float32)
    attention_mask = np.asarray(attention_mask, dtype=np.float32)
    w_pack = np.asarray(w_pack, dtype=np.float32)
    o_proj = np.asarray(o_proj, dtype=np.float32)
    gate_proj = np.asarray(gate_proj, dtype=np.float32)
    up_proj = np.asarray(up_proj, dtype=np.float32)
    down_proj = np.asarray(down_proj, dtype=np.float32)
    ln1_w = np.asarray(ln1_w, dtype=np.float32)
    ln2_w = np.asarray(ln2_w, dtype=np.float32)

    hT = np.ascontiguousarray(hidden_states.reshape(S, H).T)  # [H, S] f32
    # fold ln1 into w_pack rows; fold 1/sqrt(HD) into the q columns
    wpf = (ln1_w[:, None] * w_pack).reshape(H, 3, NH, HD).copy()
    wpf[:, 0] *= 1.0 / math.sqrt(HD)
    wgf = (ln2_w[:, None] * gate_proj).astype(NPBF16)
    wuf = (ln2_w[:, None] * up_proj).astype(NPBF16)
    wdf = down_proj.astype(NPBF16)
    mask = attention_mask.reshape(NH, S, S)

    in_maps = []
    for c in range(N_CORES):
        hsl = slice(c * NH_SH, (c + 1) * NH_SH)
        wp_sh = np.ascontiguousarray(
            wpf[:, :, hsl, :].reshape(H, 3 * F)).astype(NPBF16)
        maskT_sh = np.ascontiguousarray(
            mask[hsl].transpose(0, 2, 1))                # [5, S(j), S(i)]
        wo_sh = np.ascontiguousarray(
            o_proj[c * F:(c + 1) * F, :]).astype(NPBF16)
        wg_sh = np.ascontiguousarray(wgf[:, c * ISH:(c + 1) * ISH])
        wu_sh = np.ascontiguousarray(wuf[:, c * ISH:(c + 1) * ISH])
        wd_sh = np.ascontiguousarray(wdf[c * ISH:(c + 1) * ISH, :])
        in_maps.append({
            "hT": hT, "maskT": maskT_sh, "wp": wp_sh, "wo": wo_sh,
            "wg": wg_sh, "wu": wu_sh, "wd": wd_sh,
        })
    return in_maps


def postprocess(results):
    outT = np.empty((H, S), dtype=np.float32)
    h2_full = results[0]["h2o"]
    for c in range(N_CORES):
        outT[c * F:(c + 1) * F, :] = (
            h2_full[c * F:(c + 1) * F, :] + results[c]["out"])
    return np.ascontiguousarray(outT.T).reshape(1, S, H)


def kernel(**inputs):
    in_maps = prepare_in_maps(**inputs)
    nc = _get_nc()
    res = run_bass_kernel_spmd(nc, in_maps, list(range(N_CORES)))
    return postprocess(res.results)

